# revision 36
# baseline (speedup 1.0000x reference)
"""Trainium2 Bass kernel for BbRelProjection (per-sample QP projections).

Data-parallel over the batch: each of the 8 NeuronCores processes a
contiguous block of 524288 samples.

fp16 end-to-end on device (host casts fp32<->fp16; the rel-err budget is
2e-2 and fp16 keeps it ~1e-3): halves HBM traffic and doubles DVE
throughput (2x_1p mode for 16-bit tensor_tensor).  scalar_tensor_tensor
has no fast mode (1x), so the three scalar multiplies run on the
otherwise-idle ACT engine, issued early enough that DVE never waits.
Adjacent components are clipped in single wide [P,k,w] ops against
stride-0-broadcast bounds to amortize the ~170ns/instruction overhead;
the t-chain accumulates directly in the comp-5 slot so the ly lower clip
of (ya, yb, t) is one [P,3,w] op.

Math (exact rewrite of the reference; the nested where() collapses to a
max-chain, the isotonic pooling to min/max with the pair average):
  QP1: y0 = clip(p0, lx, ux)
  QP2: avg = 0.5*(p1+p2); y1 = clip(min(avg,p1), lx, ux)
       y2 = clip(max(avg,p2), lx, ux)
  QP3: t  = clip(max(pc, (pa+pb+pc)/3, 0.5*(pc+max(pa,pb))), ly, uy)
       ya = clip(pa, ly, t), yb = clip(pb, ly, t)
"""

import numpy as np

import concourse.bass as bass
import concourse.bacc as bacc
import concourse.mybir as mybir
from concourse.tile import TileContext
from concourse import bass_utils

N_CORES = 8
BATCH = 4194304
PER_CORE = BATCH // N_CORES  # 524288
P = 128
# Samples-per-partition for each on-device tile; sum * P == PER_CORE.
SCHEDULE = [256, 768, 1280, 1344, 448]
assert sum(SCHEDULE) * P == PER_CORE
F16 = mybir.dt.float16

MAX = mybir.AluOpType.max
MIN = mybir.AluOpType.min
ADD = mybir.AluOpType.add

ONE_THIRD = float(np.float32(1.0 / 3.0))


def build_bass() -> bass.Bass:
    nc = bacc.Bacc()
    yp = nc.dram_tensor("y_pred", [PER_CORE * 6], F16, kind="ExternalInput")
    cp = nc.dram_tensor("constr_para", [PER_CORE * 4], F16, kind="ExternalInput")
    out = nc.dram_tensor("out", [PER_CORE * 6], F16, kind="ExternalOutput")

    with TileContext(nc) as tc:
        with (
            tc.tile_pool(name="io", bufs=2) as io_pool,
            tc.tile_pool(name="tmp", bufs=2) as tmp_pool,
        ):
            # Dedicated slot per tile: all loads are emitted upfront and the
            # DMA ring streams continuously.
            tiles = []
            yoff = coff = 0
            for i, w in enumerate(SCHEDULE):
                ypt = yp[yoff : yoff + P * 6 * w].rearrange("(p c w) -> p c w", p=P, c=6)
                cpt = cp[coff : coff + P * 4 * w].rearrange("(p c w) -> p c w", p=P, c=4)
                opt = out[yoff : yoff + P * 6 * w].rearrange("(p c w) -> p c w", p=P, c=6)
                yoff += P * 6 * w
                coff += P * 4 * w

                yt = io_pool.tile([P, 6, w], F16, tag=f"yt{i}", bufs=1)
                ct = io_pool.tile([P, 4, w], F16, tag=f"ct{i}", bufs=1)
                # y first (the sums/pooling consume it immediately); bounds
                # last (first used ~9 ops into the tile).
                nc.sync.dma_start(yt[:, :, :], ypt)
                nc.sync.dma_start(ct[:, :, :], cpt)
                tiles.append((w, yt, ct, opt, ypt))

            for ti, (w, yt, ct, opt, ypt) in enumerate(tiles):
                last = ti == len(tiles) - 1
                AB = tmp_pool.tile([P, 2, w], F16, tag="AB")
                A = AB[:, 0, :]
                B = AB[:, 1, :]
                C = tmp_pool.tile([P, w], F16, tag="C")

                p = [yt[:, c, :] for c in range(6)]
                lx, ux, ly, uy = (ct[:, c, :] for c in range(4))
                lx3 = ct[:, 0:1, :].broadcast_to([P, 3, w])
                ux3 = ct[:, 1:2, :].broadcast_to([P, 3, w])
                ly3 = ct[:, 2:3, :].broadcast_to([P, 3, w])
                t2 = yt[:, 5:6, :].broadcast_to([P, 2, w])
                p012 = yt[:, 0:3, :]
                p34 = yt[:, 3:5, :]
                p345 = yt[:, 3:6, :]

                V = nc.vector
                S = nc.scalar

                # --- sums first so the ACT scalings overlap DVE work.
                # Tiles 1+: B = p3+p4+p5 is computed by chained accumulate
                # DMAs on the idle gpsimd/SWDGE queue (re-reading y from
                # DRAM), freeing 2 DVE widths.  Tile 0 keeps the DVE path
                # (its chain could not start early enough). ---
                if ti == 0:
                    V.tensor_tensor(AB, yt[:, 1:4:2, :], yt[:, 2:5:2, :], ADD)
                    S.mul(A, A, 0.5)                  # ACT: avg
                    V.tensor_tensor(B, B, p[5], ADD)
                else:
                    nc.gpsimd.dma_start(B, ypt[:, 3, :])
                    nc.gpsimd.dma_start(B, ypt[:, 4, :], accum_op=ADD)
                    nc.gpsimd.dma_start(B, ypt[:, 5, :], accum_op=ADD)
                    V.tensor_tensor(A, p[1], p[2], ADD)
                    S.mul(A, A, 0.5)                  # ACT: avg
                S.mul(B, B, ONE_THIRD)                # ACT: t_all
                V.tensor_tensor(C, p[3], p[4], MAX)
                V.tensor_tensor(C, C, p[5], ADD)
                S.mul(C, C, 0.5)                      # ACT: t_one

                # --- QP2 pool + x clips (wide over comps 0-2) ---
                V.tensor_tensor(p[1], A, p[1], MIN)
                V.tensor_tensor(p[2], A, p[2], MAX)
                V.tensor_tensor(p012, p012, lx3, MAX)
                V.tensor_tensor(p012, p012, ux3, MIN)
                nc.sync.dma_start(opt[:, 0:3, :], yt[:, 0:3, :])

                # --- QP3 t-chain; t accumulates in the comp-5 slot so the
                # ly clip covers (ya, yb, t_raw) in one wide op ---
                V.tensor_tensor(B, B, C, MAX)         # max(t_all, t_one)
                V.tensor_tensor(p[5], B, p[5], MAX)   # t_raw (>= pc)
                V.tensor_tensor(p345, p345, ly3, MAX) # lower clip pa, pb, t
                V.tensor_tensor(p[5], p[5], uy, MIN)  # t
                if last:
                    # Drain: overlap the t store with the final ya/yb op.
                    nc.sync.dma_start(opt[:, 5:6, :], yt[:, 5:6, :])
                    V.tensor_tensor(p34, p34, t2, MIN)
                    nc.sync.dma_start(opt[:, 3:5, :], yt[:, 3:5, :])
                else:
                    V.tensor_tensor(p34, p34, t2, MIN)  # ya, yb
                    nc.sync.dma_start(opt[:, 3:6, :], yt[:, 3:6, :])

    nc.finalize()
    return nc


_CACHE: dict = {}


def _get_nc() -> bass.Bass:
    if "nc" not in _CACHE:
        _CACHE["nc"] = build_bass()
    return _CACHE["nc"]


def _pack_core(x: np.ndarray, ncomp: int) -> np.ndarray:
    """[PER_CORE, ncomp] -> flat packed per SCHEDULE tiles of [P, ncomp, w]."""
    parts = []
    off = 0
    for w in SCHEDULE:
        chunk = x[off : off + P * w].reshape(P, w, ncomp)
        parts.append(chunk.transpose(0, 2, 1).reshape(-1))
        off += P * w
    return np.concatenate(parts).astype(np.float16)


def _unpack_core(x: np.ndarray, ncomp: int) -> np.ndarray:
    """Inverse of _pack_core -> [PER_CORE, ncomp]."""
    outs = []
    off = 0
    for w in SCHEDULE:
        n = P * ncomp * w
        chunk = x[off : off + n].reshape(P, ncomp, w)
        outs.append(chunk.transpose(0, 2, 1).reshape(-1, ncomp))
        off += n
    return np.concatenate(outs).astype(np.float32)


def make_in_maps(y_pred: np.ndarray, constr_para: np.ndarray):
    y = np.ascontiguousarray(y_pred, dtype=np.float32)
    c = np.ascontiguousarray(constr_para, dtype=np.float32)
    return [
        {
            "y_pred": _pack_core(y[i * PER_CORE : (i + 1) * PER_CORE], 6),
            "constr_para": _pack_core(c[i * PER_CORE : (i + 1) * PER_CORE], 4),
        }
        for i in range(N_CORES)
    ]


def gather_out(results) -> np.ndarray:
    return np.concatenate(
        [_unpack_core(results[i]["out"], 6) for i in range(N_CORES)], axis=0
    )


def run_sharded(y_pred: np.ndarray, constr_para: np.ndarray, **spmd_kwargs):
    """Shard over 8 cores, run, and return (full_output, BassKernelResults)."""
    nc = _get_nc()
    in_maps = make_in_maps(y_pred, constr_para)
    res = bass_utils.run_bass_kernel_spmd(nc, in_maps, list(range(N_CORES)), **spmd_kwargs)
    return gather_out(res.results), res


def kernel(y_pred: np.ndarray, constr_para: np.ndarray) -> np.ndarray:
    assert y_pred.shape == (BATCH, 6) and constr_para.shape == (BATCH, 4)
    full, _ = run_sharded(y_pred, constr_para)
    return full


# revision 37
# speedup vs baseline: 1.3600x; 1.3600x over previous
"""Trainium2 Bass kernel for BbRelProjection (per-sample QP projections).

Data-parallel over the batch: each of the 8 NeuronCores processes a
contiguous block of 524288 samples.

fp16 end-to-end on device (host casts fp32<->fp16; the rel-err budget is
2e-2 and fp16 keeps it ~1e-3): halves HBM traffic and doubles DVE
throughput (2x_1p mode for 16-bit tensor_tensor).  scalar_tensor_tensor
has no fast mode (1x), so the three scalar multiplies run on the
otherwise-idle ACT engine, issued early enough that DVE never waits.
Adjacent components are clipped in single wide [P,k,w] ops against
stride-0-broadcast bounds to amortize the ~170ns/instruction overhead;
the t-chain accumulates directly in the comp-5 slot so the ly lower clip
of (ya, yb, t) is one [P,3,w] op.

Math (exact rewrite of the reference; the nested where() collapses to a
max-chain, the isotonic pooling to min/max with the pair average):
  QP1: y0 = clip(p0, lx, ux)
  QP2: avg = 0.5*(p1+p2); y1 = clip(min(avg,p1), lx, ux)
       y2 = clip(max(avg,p2), lx, ux)
  QP3: t  = clip(max(pc, (pa+pb+pc)/3, 0.5*(pc+max(pa,pb))), ly, uy)
       ya = clip(pa, ly, t), yb = clip(pb, ly, t)
"""

import numpy as np

import concourse.bass as bass
import concourse.bacc as bacc
import concourse.mybir as mybir
from concourse.tile import TileContext
from concourse import bass_utils

N_CORES = 8
BATCH = 4194304
PER_CORE = BATCH // N_CORES  # 524288
P = 128
# Samples-per-partition for each on-device tile; sum * P == PER_CORE.
SCHEDULE = [256, 768, 1280, 1344, 448]
assert sum(SCHEDULE) * P == PER_CORE
F16 = mybir.dt.float16

MAX = mybir.AluOpType.max
MIN = mybir.AluOpType.min
ADD = mybir.AluOpType.add

ONE_THIRD = float(np.float32(1.0 / 3.0))


def build_bass() -> bass.Bass:
    nc = bacc.Bacc()
    yp = nc.dram_tensor("y_pred", [PER_CORE * 6], F16, kind="ExternalInput")
    cp = nc.dram_tensor("constr_para", [PER_CORE * 4], F16, kind="ExternalInput")
    out = nc.dram_tensor("out", [PER_CORE * 6], F16, kind="ExternalOutput")

    with TileContext(nc) as tc:
        with (
            tc.tile_pool(name="io", bufs=2) as io_pool,
            tc.tile_pool(name="tmp", bufs=2) as tmp_pool,
        ):
            # Dedicated slot per tile: all loads are emitted upfront and the
            # DMA ring streams continuously.
            tiles = []
            yoff = coff = 0
            for i, w in enumerate(SCHEDULE):
                ypt = yp[yoff : yoff + P * 6 * w].rearrange("(p c w) -> p c w", p=P, c=6)
                cpt = cp[coff : coff + P * 4 * w].rearrange("(p c w) -> p c w", p=P, c=4)
                opt = out[yoff : yoff + P * 6 * w].rearrange("(p c w) -> p c w", p=P, c=6)
                yoff += P * 6 * w
                coff += P * 4 * w

                yt = io_pool.tile([P, 6, w], F16, tag=f"yt{i}", bufs=1)
                ct = io_pool.tile([P, 4, w], F16, tag=f"ct{i}", bufs=1)
                # y first (the sums/pooling consume it immediately); bounds
                # last (first used ~9 ops into the tile).
                nc.sync.dma_start(yt[:, :, :], ypt)
                nc.sync.dma_start(ct[:, :, :], cpt)
                tiles.append((w, yt, ct, opt))

            for ti, (w, yt, ct, opt) in enumerate(tiles):
                last = ti == len(tiles) - 1
                AB = tmp_pool.tile([P, 2, w], F16, tag="AB")
                A = AB[:, 0, :]
                B = AB[:, 1, :]
                C = tmp_pool.tile([P, w], F16, tag="C")

                p = [yt[:, c, :] for c in range(6)]
                lx, ux, ly, uy = (ct[:, c, :] for c in range(4))
                lx3 = ct[:, 0:1, :].broadcast_to([P, 3, w])
                ux3 = ct[:, 1:2, :].broadcast_to([P, 3, w])
                ly3 = ct[:, 2:3, :].broadcast_to([P, 3, w])
                t2 = yt[:, 5:6, :].broadcast_to([P, 2, w])
                p012 = yt[:, 0:3, :]
                p34 = yt[:, 3:5, :]
                p345 = yt[:, 3:6, :]

                V = nc.vector
                S = nc.scalar

                # --- sums first so the ACT scalings overlap DVE work;
                # A=p1+p2 and B=p3+p4 fuse into one strided wide add ---
                V.tensor_tensor(AB, yt[:, 1:4:2, :], yt[:, 2:5:2, :], ADD)
                S.mul(A, A, 0.5)                      # ACT: avg
                V.tensor_tensor(B, B, p[5], ADD)
                S.mul(B, B, ONE_THIRD)                # ACT: t_all
                V.tensor_tensor(C, p[3], p[4], MAX)
                V.tensor_tensor(C, C, p[5], ADD)
                S.mul(C, C, 0.5)                      # ACT: t_one

                # --- QP2 pool + x clips (wide over comps 0-2) ---
                V.tensor_tensor(p[1], A, p[1], MIN)
                V.tensor_tensor(p[2], A, p[2], MAX)
                V.tensor_tensor(p012, p012, lx3, MAX)
                V.tensor_tensor(p012, p012, ux3, MIN)
                nc.sync.dma_start(opt[:, 0:3, :], yt[:, 0:3, :])

                # --- QP3 t-chain; t accumulates in the comp-5 slot so the
                # ly clip covers (ya, yb, t_raw) in one wide op ---
                V.tensor_tensor(B, B, C, MAX)         # max(t_all, t_one)
                V.tensor_tensor(p[5], B, p[5], MAX)   # t_raw (>= pc)
                V.tensor_tensor(p345, p345, ly3, MAX) # lower clip pa, pb, t
                V.tensor_tensor(p[5], p[5], uy, MIN)  # t
                if last:
                    # Drain: overlap the t store with the final ya/yb op.
                    nc.sync.dma_start(opt[:, 5:6, :], yt[:, 5:6, :])
                    V.tensor_tensor(p34, p34, t2, MIN)
                    nc.sync.dma_start(opt[:, 3:5, :], yt[:, 3:5, :])
                else:
                    V.tensor_tensor(p34, p34, t2, MIN)  # ya, yb
                    nc.sync.dma_start(opt[:, 3:6, :], yt[:, 3:6, :])

    nc.finalize()
    return nc


_CACHE: dict = {}


def _get_nc() -> bass.Bass:
    if "nc" not in _CACHE:
        _CACHE["nc"] = build_bass()
    return _CACHE["nc"]


def _pack_core(x: np.ndarray, ncomp: int) -> np.ndarray:
    """[PER_CORE, ncomp] -> flat packed per SCHEDULE tiles of [P, ncomp, w]."""
    parts = []
    off = 0
    for w in SCHEDULE:
        chunk = x[off : off + P * w].reshape(P, w, ncomp)
        parts.append(chunk.transpose(0, 2, 1).reshape(-1))
        off += P * w
    return np.concatenate(parts).astype(np.float16)


def _unpack_core(x: np.ndarray, ncomp: int) -> np.ndarray:
    """Inverse of _pack_core -> [PER_CORE, ncomp]."""
    outs = []
    off = 0
    for w in SCHEDULE:
        n = P * ncomp * w
        chunk = x[off : off + n].reshape(P, ncomp, w)
        outs.append(chunk.transpose(0, 2, 1).reshape(-1, ncomp))
        off += n
    return np.concatenate(outs).astype(np.float32)


def make_in_maps(y_pred: np.ndarray, constr_para: np.ndarray):
    y = np.ascontiguousarray(y_pred, dtype=np.float32)
    c = np.ascontiguousarray(constr_para, dtype=np.float32)
    return [
        {
            "y_pred": _pack_core(y[i * PER_CORE : (i + 1) * PER_CORE], 6),
            "constr_para": _pack_core(c[i * PER_CORE : (i + 1) * PER_CORE], 4),
        }
        for i in range(N_CORES)
    ]


def gather_out(results) -> np.ndarray:
    return np.concatenate(
        [_unpack_core(results[i]["out"], 6) for i in range(N_CORES)], axis=0
    )


def run_sharded(y_pred: np.ndarray, constr_para: np.ndarray, **spmd_kwargs):
    """Shard over 8 cores, run, and return (full_output, BassKernelResults)."""
    nc = _get_nc()
    in_maps = make_in_maps(y_pred, constr_para)
    res = bass_utils.run_bass_kernel_spmd(nc, in_maps, list(range(N_CORES)), **spmd_kwargs)
    return gather_out(res.results), res


def kernel(y_pred: np.ndarray, constr_para: np.ndarray) -> np.ndarray:
    assert y_pred.shape == (BATCH, 6) and constr_para.shape == (BATCH, 4)
    full, _ = run_sharded(y_pred, constr_para)
    return full
